# revision 22
# baseline (speedup 1.0000x reference)
"""CGNN message-passing kernel for 8 trn2 NeuronCores.

Algorithm (per image (b,a), image = [S=768, T=14] grid):
  x = pw_vh(dw_hh(concat(h2,h1))) + pw_vp(dw_hp(pe)) + beta   (conv1 + pe branch)
  x = relu(x)
  y = pw_ov(dw_oh(x)) + beta2                                 (conv2)

Layout strategy: channel-major SBUF tiles [(chan,t) partitions, s free], bf16
matmul data with fp32 PSUM accumulation.

conv1: depthwise 3x3 becomes 3 accumulating matmuls (one per s-shift ds) with
host-precomputed banded lhsT matrices that encode the t-direction taps
(T=14 blocks on the partition axis). The pointwise 6->32 conv, the pe-branch
pointwise 2->32, and the total bias are stacked into one K=113 matmul
(rhs = [hd(84 rows); pe_dw(28); ones(1)]).

conv2 (fused dw_oh+pw_ov banded weights) runs "data-stationary": lhsT is a
128-column window of x, rhs is the small [K,28] weight block, so the output
lands PIXEL-major ([s partitions, (k,t) free]) and needs no egress transposes
— one DVE add(bias)+copy per antenna feeds the store tile directly.

Sharding: data-parallel over batch B=16 -> 2 batches per core. Inputs are
cast to bf16 host-side (also halves HBM ingest traffic).
"""

import numpy as np
import ml_dtypes
from contextlib import ExitStack

import concourse.bass as bass
import concourse.bacc as bacc
import concourse.tile as tile
from concourse import mybir
from concourse.bass_utils import run_bass_kernel_spmd

F32 = mybir.dt.float32
BF16 = mybir.dt.bfloat16
NPBF16 = ml_dtypes.bfloat16
B, S, T, A = 16, 768, 14, 16
HK0, PEK0, U, K1 = 6, 2, 32, 2
NCORES = 8
BPC = B // NCORES          # batches per core
NST = S // 128             # 6 s-tiles of 128
SP = S + 2                 # s-padded width (zero col at 0 and S+1)
UCH = [9, 9, 9, 5]         # u-chunk sizes (32 = 9+9+9+5)
UOF = [0, 9, 18, 27]
SCH = [(0, 384), (384, 384)]   # s chunks (PSUM bank = 512 fp32 max)
KPW = 6 * T + 2 * T + 1        # 113: stacked K for the conv1 pointwise


def _tband(w_t, n_t=T):
    """[n_t, n_t] band matrix M[t, t'] = w_t[t - t' + 1] (3-tap, SAME pad)."""
    m = np.zeros((n_t, n_t), np.float32)
    for t in range(n_t):
        for tp in range(n_t):
            dt = t - tp + 1
            if 0 <= dt <= 2:
                m[t, tp] = w_t[dt]
    return m


def build_consts(w_hh, b_hh, w_vh, b_vh, w_hp, b_hp, w_vp, b_vp,
                 w_oh, b_oh, w_ov, b_ov):
    """Host-side precompute of all lhsT matrices. Returns dict name->array."""
    w_hh = w_hh[:, :, 0, :]   # [3,3,6]
    w_hp = w_hp[:, :, 0, :]   # [3,3,2]
    w_oh = w_oh[:, :, 0, :]   # [3,3,32]

    # conv1 depthwise band: [3, 84, 84], rows/cols = g*14+t, g = concat chan
    B1 = np.zeros((3, 6 * T, 6 * T), np.float32)
    for ds in range(3):
        for g in range(6):
            B1[ds, g * T:(g + 1) * T, g * T:(g + 1) * T] = _tband(w_hh[ds, :, g])

    # conv1 pointwise, stacked K = [hd(84); pe_dw(28); ones(1)]: [113, 448]
    # cols = concat over uc of (u_local, t')
    beta = (b_vh + w_vh.T @ b_hh + b_vp + w_vp.T @ b_hp).astype(np.float32)  # [32]
    W1s = np.zeros((KPW, sum(u * T for u in UCH)), np.float32)
    col = 0
    for uc in range(4):
        for ul in range(UCH[uc]):
            u = UOF[uc] + ul
            for g in range(6):
                W1s[g * T:(g + 1) * T, col:col + T] = np.eye(T, dtype=np.float32) * w_vh[g, u]
            for c in range(2):
                W1s[(6 + c) * T:(7 + c) * T, col:col + T] = np.eye(T, dtype=np.float32) * w_vp[c, u]
            W1s[8 * T, col:col + T] = beta[u]
            col += T

    # pe depthwise band: [3, 28, 28]
    Bpe = np.zeros((3, 2 * T, 2 * T), np.float32)
    for ds in range(3):
        for c in range(2):
            Bpe[ds, c * T:(c + 1) * T, c * T:(c + 1) * T] = _tband(w_hp[ds, :, c])

    # conv2 fused band (dw_oh folded with pw_ov): [126, 12*28]
    # col block (uc*3+ds): [uch*14, 28]; rows (u_local,t'), cols (k,t)
    B2 = np.zeros((9 * T, 12 * (K1 * T)), np.float32)
    for uc in range(4):
        for ds in range(3):
            blk = np.zeros((UCH[uc] * T, K1 * T), np.float32)
            for ul in range(UCH[uc]):
                u = UOF[uc] + ul
                band = _tband(w_oh[ds, :, u])          # [T, T]
                for k in range(K1):
                    blk[ul * T:(ul + 1) * T, k * T:(k + 1) * T] = band * w_ov[u, k]
            c0 = (uc * 3 + ds) * (K1 * T)
            B2[:UCH[uc] * T, c0:c0 + K1 * T] = blk

    # conv2 bias: ones-row trick -- x chunk uc=3 carries a constant-1 row at
    # local row 70, and B2's (uc=3, ds=1) block row 70 holds the bias pattern
    beta2 = (b_ov + w_ov.T @ b_oh).astype(np.float32)  # [2]
    B2[UCH[3] * T, (3 * 3 + 1) * (K1 * T):(3 * 3 + 2) * (K1 * T)] = \
        np.repeat(beta2, T)

    return {
        "wI": np.eye(128, dtype=np.float32),
        "wB1": B1, "wW1s": W1s, "wBpe": Bpe, "wB2": B2,
    }


def _trace_kernel(nc):
    h1 = nc.dram_tensor("h1", [BPC, S, T, A, 3], BF16, kind="ExternalInput").ap()
    h2 = nc.dram_tensor("h2", [BPC, S, T, A, 3], BF16, kind="ExternalInput").ap()
    pe = nc.dram_tensor("pe", [BPC, S, T, PEK0], BF16, kind="ExternalInput").ap()
    wI = nc.dram_tensor("wI", [128, 128], BF16, kind="ExternalInput").ap()
    wB1 = nc.dram_tensor("wB1", [3, 84, 84], BF16, kind="ExternalInput").ap()
    wW1s = nc.dram_tensor("wW1s", [KPW, 448], BF16, kind="ExternalInput").ap()
    wBpe = nc.dram_tensor("wBpe", [3, 28, 28], BF16, kind="ExternalInput").ap()
    wB2 = nc.dram_tensor("wB2", [126, 336], BF16, kind="ExternalInput").ap()
    y = nc.dram_tensor("y", [BPC, S, T, A, K1], BF16, kind="ExternalOutput").ap()

    RELU = mybir.ActivationFunctionType.Relu
    ADD = mybir.AluOpType.add

    with tile.TileContext(nc) as tc, ExitStack() as ctx:
        wp = ctx.enter_context(tc.tile_pool(name="w", bufs=1))
        hstage_p = ctx.enter_context(tc.tile_pool(name="hstage", bufs=2))
        pestage_p = ctx.enter_context(tc.tile_pool(name="pestage", bufs=2))
        ht_p = ctx.enter_context(tc.tile_pool(name="ht", bufs=2))
        hdp_p = ctx.enter_context(tc.tile_pool(name="hdp", bufs=2))
        pedw_p = ctx.enter_context(tc.tile_pool(name="pedw", bufs=2))
        x_p = ctx.enter_context(tc.tile_pool(name="xs", bufs=2))
        hcp_p = ctx.enter_context(tc.tile_pool(name="hcp", bufs=2))
        pcp_p = ctx.enter_context(tc.tile_pool(name="pcp", bufs=2))
        stout_p = ctx.enter_context(tc.tile_pool(name="stout", bufs=2))
        ptp = ctx.enter_context(tc.tile_pool(name="ptp", bufs=2, space="PSUM"))
        pdw = ctx.enter_context(tc.tile_pool(name="pdw", bufs=2, space="PSUM"))
        px = ctx.enter_context(tc.tile_pool(name="px", bufs=2, space="PSUM"))
        py = ctx.enter_context(tc.tile_pool(name="py", bufs=2, space="PSUM"))

        ident = wp.tile([128, 128], BF16)
        nc.gpsimd.dma_start(ident[:], wI)
        b1 = wp.tile([84, 3, 84], BF16)
        nc.gpsimd.dma_start(b1[:], wB1.rearrange("d k m -> k d m"))
        w1s = wp.tile([KPW, 448], BF16)
        nc.gpsimd.dma_start(w1s[:], wW1s)
        bpe = wp.tile([28, 3, 28], BF16)
        nc.gpsimd.dma_start(bpe[:], wBpe.rearrange("d k m -> k d m"))
        b2t = wp.tile([126, 336], BF16)
        nc.gpsimd.dma_start(b2t[:], wB2)

        # spin the PE while the first activations stream in, so the HAM
        # clock gate is released by the time real matmuls arrive
        warm = px.tile([126, 384], F32, tag="xq")
        for _ in range(30):
            nc.tensor.matmul(warm[:, 0:128], ident[:, 0:126], ident[:],
                             start=True, stop=True)

        for b in range(BPC):
            hs = hstage_p.tile([128, 2, NST, T, A, 3], BF16, tag="hs")
            nc.sync.dma_start(hs[:, 0], h2[b].rearrange("(n p) t a c -> p n t a c", p=128))
            nc.sync.dma_start(hs[:, 1], h1[b].rearrange("(n p) t a c -> p n t a c", p=128))
            ps = pestage_p.tile([128, NST, T, PEK0], BF16, tag="ps")
            nc.sync.dma_start(ps[:], pe[b].rearrange("(n p) t c -> p n t c", p=128))

            # ---- pe branch (per b, shared by all 16 antennas) ----
            pcp = pcp_p.tile([128, NST, PEK0, T], BF16, tag="pcp")
            nc.vector.tensor_copy(pcp[:], ps[:].rearrange("p n t c -> p n c t"))
            peT = ht_p.tile([28, SP], BF16, tag="peT")
            nc.vector.memset(peT[:], 0.0)
            for g in range(2):
                tp = ptp.tile([84, 384], BF16, tag="tp")
                for j in range(3):
                    st = g * 3 + j
                    nc.tensor.transpose(
                        tp[0:28, j * 128:(j + 1) * 128],
                        pcp[:, st], ident[:])
                nc.vector.tensor_copy(peT[0:28, 1 + g * 384:1 + (g + 1) * 384], tp[0:28, :])
            # pe_dw rows [29]: rows 0:28 = depthwise(pe), row 28 = ones
            pedw = pedw_p.tile([29, S], BF16, tag="pedw")
            nc.vector.memset(pedw[:], 1.0)
            for g, (s0, sn) in enumerate(SCH):
                dq = pdw.tile([84, 384], F32, tag="dq")
                for ds in range(3):
                    nc.tensor.matmul(dq[0:28, :sn], bpe[:, ds, :],
                                     peT[:, ds + s0: ds + s0 + sn],
                                     start=(ds == 0), stop=(ds == 2))
                nc.scalar.copy(pedw[0:28, s0:s0 + sn], dq[0:28, :sn])

            # two persistent conv1-pw rhs tiles (double-buffered by antenna
            # parity); rows 84:113 = pe branch, written once per b
            hdps = [hdp_p.tile([KPW, S], BF16, tag=f"hdp{i}", name=f"hdp{i}")
                    for i in range(2)]
            for i in range(2):
                nc.sync.dma_start(hdps[i][84:KPW, :], pedw[:])

            # persistent hT / x tiles (halos zeroed once per b; x chunk uc=3
            # has an extra all-ones row 70 feeding the conv2 bias)
            hTs_ = []
            xss_ = []
            for i in range(2):
                hT = ht_p.tile([84, SP], BF16, tag=f"hT{i}", name=f"hT{i}")
                nc.vector.memset(hT[:, 0:1], 0.0)
                nc.vector.memset(hT[:, SP - 1:SP], 0.0)
                hTs_.append(hT)
                xs = []
                for uc in range(4):
                    m = UCH[uc] * T + (1 if uc == 3 else 0)
                    xt = x_p.tile([m, SP], BF16, tag=f"x{uc}_{i}",
                                  name=f"x{uc}_{i}")
                    if uc == 3:
                        nc.vector.memset(xt[:], 1.0)
                    nc.vector.memset(xt[:, 0:1], 0.0)
                    nc.vector.memset(xt[:, SP - 1:SP], 0.0)
                    xs.append(xt)
                xss_.append(xs)

            stout = stout_p.tile([128, NST, T, A, K1], BF16, tag="so")

            # ---- per-antenna images, processed in interleaved pairs ----
            def ingest(a):
                hcp = hcp_p.tile([128, NST, 2, 3, T], BF16, tag=f"hcp{a % 2}")
                for h in range(2):
                    src = hs[:, h, :, :, a, :].rearrange("p n t c -> p n c t")
                    nc.gpsimd.tensor_copy(hcp[:, :, h], src)
                hT = hTs_[a % 2]
                for g in range(2):
                    tp = ptp.tile([84, 384], BF16, tag="tp")
                    for j in range(3):
                        st = g * 3 + j
                        nc.tensor.transpose(
                            tp[:, j * 128:(j + 1) * 128],
                            hcp[:, st], ident[:])
                    dst = hT[:, 1 + g * 384:1 + (g + 1) * 384]
                    if g == 0:
                        nc.vector.tensor_copy(dst, tp[:, :])
                    else:
                        nc.scalar.copy(dst, tp[:, :])
                return hT

            def dw1(a, hT):
                hdp = hdps[a % 2]
                for g, (s0, sn) in enumerate(SCH):
                    dq = pdw.tile([84, 384], F32, tag="dq")
                    for ds in range(3):
                        nc.tensor.matmul(dq[:, :sn], b1[:, ds, :],
                                         hT[:, ds + s0: ds + s0 + sn],
                                         start=(ds == 0), stop=(ds == 2))
                    if g == 0:
                        nc.vector.tensor_copy(hdp[0:84, s0:s0 + sn], dq[:, :sn])
                    else:
                        nc.scalar.copy(hdp[0:84, s0:s0 + sn], dq[:, :sn])
                return hdp

            def pw_relu(a, hdp):
                xs = xss_[a % 2]
                for uc in range(4):
                    m = UCH[uc] * T
                    c0 = UOF[uc] * T
                    xt = xs[uc]
                    for g, (s0, sn) in enumerate(SCH):
                        xq = px.tile([126, 384], F32, tag="xq")
                        nc.tensor.matmul(xq[0:m, :sn], w1s[:, c0:c0 + m],
                                         hdp[:, s0:s0 + sn], start=True, stop=True)
                        dst = xt[0:m, 1 + s0:1 + s0 + sn]
                        if (uc * 2 + g + a) % 2 == 0:
                            nc.scalar.activation(dst, xq[0:m, :sn], RELU)
                        else:
                            nc.vector.tensor_scalar_max(dst, xq[0:m, :sn], 0.0)
                return xs

            def conv2(a, xs):
                y2 = py.tile([128, NST, K1 * T], F32, tag="y2")
                for n in range(NST):
                    kk = 0
                    for uc in range(4):
                        m = UCH[uc] * T + (1 if uc == 3 else 0)
                        for ds in range(3):
                            c0 = (uc * 3 + ds) * (K1 * T)
                            nc.tensor.matmul(
                                y2[:, n],
                                xs[uc][0:m, n * 128 + ds: n * 128 + ds + 128],
                                b2t[0:m, c0:c0 + K1 * T],
                                start=(kk == 0), stop=(kk == 11))
                            kk += 1
                # reorder (k,t)->(t,k) into the store tile (bias already in)
                src_ = y2[:].rearrange("p n (k t) -> p n t k", k=K1)
                if a % 2 == 0:
                    nc.scalar.copy(stout[:, :, :, a, :], src_)
                else:
                    nc.vector.tensor_copy(stout[:, :, :, a, :], src_)

            for half in range(A // 2):
                pair = (2 * half, 2 * half + 1)
                hTs = {a: ingest(a) for a in pair}
                hd_ = {a: dw1(a, hTs[a]) for a in pair}
                xs_ = {a: pw_relu(a, hd_[a]) for a in pair}
                for a in pair:
                    conv2(a, xs_[a])

            yv = y[b].rearrange("(n p) t a k -> p n t a k", p=128)
            nc.sync.dma_start(yv[:, 0:3], stout[:, 0:3])
            nc.scalar.dma_start(yv[:, 3:6], stout[:, 3:6])
    nc.compile()
    return nc


_CACHED_NC = None


def get_nc():
    global _CACHED_NC
    if _CACHED_NC is None:
        _CACHED_NC = _trace_kernel(
            bacc.Bacc("TRN2", target_bir_lowering=False, debug=False))
    return _CACHED_NC


def make_in_maps(inputs):
    consts = build_consts(
        inputs["w_hh"], inputs["b_hh"], inputs["w_vh"], inputs["b_vh"],
        inputs["w_hp"], inputs["b_hp"], inputs["w_vp"], inputs["b_vp"],
        inputs["w_oh"], inputs["b_oh"], inputs["w_ov"], inputs["b_ov"])
    consts = {
        k: np.ascontiguousarray(v, NPBF16) for k, v in consts.items()
    }
    in_maps = []
    for i in range(NCORES):
        sl = slice(i * BPC, (i + 1) * BPC)
        m = {
            "h1": np.ascontiguousarray(inputs["h1"][sl], NPBF16),
            "h2": np.ascontiguousarray(inputs["h2"][sl], NPBF16),
            "pe": np.ascontiguousarray(inputs["pe"][sl], NPBF16),
        }
        m.update(consts)
        in_maps.append(m)
    return in_maps


def kernel(**inputs):
    nc = get_nc()
    in_maps = make_in_maps(inputs)
    res = run_bass_kernel_spmd(nc, in_maps, list(range(NCORES)))
    return np.concatenate([r["y"] for r in res.results], axis=0).astype(np.float32)


# revision 24
# speedup vs baseline: 1.0075x; 1.0075x over previous
"""CGNN message-passing kernel for 8 trn2 NeuronCores.

Algorithm (per image (b,a), image = [S=768, T=14] grid):
  x = pw_vh(dw_hh(concat(h2,h1))) + pw_vp(dw_hp(pe)) + beta   (conv1 + pe branch)
  x = relu(x)
  y = pw_ov(dw_oh(x)) + beta2                                 (conv2)

Layout strategy: channel-major SBUF tiles [(chan,t) partitions, s free], bf16
matmul data with fp32 PSUM accumulation.

conv1: depthwise 3x3 becomes 3 accumulating matmuls (one per s-shift ds) with
host-precomputed banded lhsT matrices that encode the t-direction taps
(T=14 blocks on the partition axis). The pointwise 6->32 conv, the pe-branch
pointwise 2->32, and the total bias are stacked into one K=113 matmul
(rhs = [hd(84 rows); pe_dw(28); ones(1)]).

conv2 (fused dw_oh+pw_ov banded weights) runs "data-stationary": lhsT is a
128-column window of x, rhs is the small [K,28] weight block, so the output
lands PIXEL-major ([s partitions, (k,t) free]) and needs no egress transposes
— one DVE add(bias)+copy per antenna feeds the store tile directly.

Sharding: data-parallel over batch B=16 -> 2 batches per core. Inputs are
cast to bf16 host-side (also halves HBM ingest traffic).
"""

import numpy as np
import ml_dtypes
from contextlib import ExitStack

import concourse.bass as bass
import concourse.bacc as bacc
import concourse.tile as tile
from concourse import mybir
from concourse.bass_utils import run_bass_kernel_spmd

F32 = mybir.dt.float32
BF16 = mybir.dt.bfloat16
NPBF16 = ml_dtypes.bfloat16
B, S, T, A = 16, 768, 14, 16
HK0, PEK0, U, K1 = 6, 2, 32, 2
NCORES = 8
BPC = B // NCORES          # batches per core
NST = S // 128             # 6 s-tiles of 128
SP = S + 2                 # s-padded width (zero col at 0 and S+1)
UCH = [9, 9, 9, 5]         # u-chunk sizes (32 = 9+9+9+5)
UOF = [0, 9, 18, 27]
SCH = [(0, 384), (384, 384)]   # s chunks (PSUM bank = 512 fp32 max)
KPW = 6 * T + 2 * T + 1        # 113: stacked K for the conv1 pointwise


def _tband(w_t, n_t=T):
    """[n_t, n_t] band matrix M[t, t'] = w_t[t - t' + 1] (3-tap, SAME pad)."""
    m = np.zeros((n_t, n_t), np.float32)
    for t in range(n_t):
        for tp in range(n_t):
            dt = t - tp + 1
            if 0 <= dt <= 2:
                m[t, tp] = w_t[dt]
    return m


def build_consts(w_hh, b_hh, w_vh, b_vh, w_hp, b_hp, w_vp, b_vp,
                 w_oh, b_oh, w_ov, b_ov):
    """Host-side precompute of all lhsT matrices. Returns dict name->array."""
    w_hh = w_hh[:, :, 0, :]   # [3,3,6]
    w_hp = w_hp[:, :, 0, :]   # [3,3,2]
    w_oh = w_oh[:, :, 0, :]   # [3,3,32]

    # conv1 depthwise band: [3, 84, 84], rows/cols = g*14+t, g = concat chan
    B1 = np.zeros((3, 6 * T, 6 * T), np.float32)
    for ds in range(3):
        for g in range(6):
            B1[ds, g * T:(g + 1) * T, g * T:(g + 1) * T] = _tband(w_hh[ds, :, g])

    # conv1 pointwise, stacked K = [hd(84); pe_dw(28); ones(1)]: [113, 448]
    # cols = concat over uc of (u_local, t')
    beta = (b_vh + w_vh.T @ b_hh + b_vp + w_vp.T @ b_hp).astype(np.float32)  # [32]
    W1s = np.zeros((KPW, sum(u * T for u in UCH)), np.float32)
    col = 0
    for uc in range(4):
        for ul in range(UCH[uc]):
            u = UOF[uc] + ul
            for g in range(6):
                W1s[g * T:(g + 1) * T, col:col + T] = np.eye(T, dtype=np.float32) * w_vh[g, u]
            for c in range(2):
                W1s[(6 + c) * T:(7 + c) * T, col:col + T] = np.eye(T, dtype=np.float32) * w_vp[c, u]
            W1s[8 * T, col:col + T] = beta[u]
            col += T

    # pe depthwise band: [3, 28, 28]
    Bpe = np.zeros((3, 2 * T, 2 * T), np.float32)
    for ds in range(3):
        for c in range(2):
            Bpe[ds, c * T:(c + 1) * T, c * T:(c + 1) * T] = _tband(w_hp[ds, :, c])

    # conv2 fused band (dw_oh folded with pw_ov): [126, 12*28]
    # col block (uc*3+ds): [uch*14, 28]; rows (u_local,t'), cols (k,t)
    B2 = np.zeros((9 * T, 12 * (K1 * T)), np.float32)
    for uc in range(4):
        for ds in range(3):
            blk = np.zeros((UCH[uc] * T, K1 * T), np.float32)
            for ul in range(UCH[uc]):
                u = UOF[uc] + ul
                band = _tband(w_oh[ds, :, u])          # [T, T]
                for k in range(K1):
                    blk[ul * T:(ul + 1) * T, k * T:(k + 1) * T] = band * w_ov[u, k]
            c0 = (uc * 3 + ds) * (K1 * T)
            B2[:UCH[uc] * T, c0:c0 + K1 * T] = blk

    # conv2 bias: ones-row trick -- x chunk uc=3 carries a constant-1 row at
    # local row 70, and B2's (uc=3, ds=1) block row 70 holds the bias pattern
    beta2 = (b_ov + w_ov.T @ b_oh).astype(np.float32)  # [2]
    B2[UCH[3] * T, (3 * 3 + 1) * (K1 * T):(3 * 3 + 2) * (K1 * T)] = \
        np.repeat(beta2, T)

    return {
        "wI": np.eye(128, dtype=np.float32),
        "wB1": B1, "wW1s": W1s, "wBpe": Bpe, "wB2": B2,
    }


def _trace_kernel(nc):
    h1 = nc.dram_tensor("h1", [BPC, S, T, A, 3], BF16, kind="ExternalInput").ap()
    h2 = nc.dram_tensor("h2", [BPC, S, T, A, 3], BF16, kind="ExternalInput").ap()
    pe = nc.dram_tensor("pe", [BPC, S, T, PEK0], BF16, kind="ExternalInput").ap()
    wI = nc.dram_tensor("wI", [128, 128], BF16, kind="ExternalInput").ap()
    wB1 = nc.dram_tensor("wB1", [3, 84, 84], BF16, kind="ExternalInput").ap()
    wW1s = nc.dram_tensor("wW1s", [KPW, 448], BF16, kind="ExternalInput").ap()
    wBpe = nc.dram_tensor("wBpe", [3, 28, 28], BF16, kind="ExternalInput").ap()
    wB2 = nc.dram_tensor("wB2", [126, 336], BF16, kind="ExternalInput").ap()
    y = nc.dram_tensor("y", [BPC, S, T, A, K1], BF16, kind="ExternalOutput").ap()

    RELU = mybir.ActivationFunctionType.Relu
    ADD = mybir.AluOpType.add

    with tile.TileContext(nc) as tc, ExitStack() as ctx:
        wp = ctx.enter_context(tc.tile_pool(name="w", bufs=1))
        hstage_p = ctx.enter_context(tc.tile_pool(name="hstage", bufs=2))
        pestage_p = ctx.enter_context(tc.tile_pool(name="pestage", bufs=2))
        ht_p = ctx.enter_context(tc.tile_pool(name="ht", bufs=2))
        hdp_p = ctx.enter_context(tc.tile_pool(name="hdp", bufs=2))
        pedw_p = ctx.enter_context(tc.tile_pool(name="pedw", bufs=2))
        x_p = ctx.enter_context(tc.tile_pool(name="xs", bufs=2))
        hcp_p = ctx.enter_context(tc.tile_pool(name="hcp", bufs=2))
        pcp_p = ctx.enter_context(tc.tile_pool(name="pcp", bufs=2))
        stout_p = ctx.enter_context(tc.tile_pool(name="stout", bufs=2))
        ptp = ctx.enter_context(tc.tile_pool(name="ptp", bufs=2, space="PSUM"))
        pdw = ctx.enter_context(tc.tile_pool(name="pdw", bufs=2, space="PSUM"))
        px = ctx.enter_context(tc.tile_pool(name="px", bufs=2, space="PSUM"))
        py = ctx.enter_context(tc.tile_pool(name="py", bufs=2, space="PSUM"))

        ident = wp.tile([128, 128], BF16)
        nc.scalar.dma_start(ident[:], wI)
        b1 = wp.tile([84, 3, 84], BF16)
        nc.scalar.dma_start(b1[:], wB1.rearrange("d k m -> k d m"))
        w1s = wp.tile([KPW, 448], BF16)
        nc.scalar.dma_start(w1s[:], wW1s)
        bpe = wp.tile([28, 3, 28], BF16)
        nc.scalar.dma_start(bpe[:], wBpe.rearrange("d k m -> k d m"))
        b2t = wp.tile([126, 336], BF16)
        nc.scalar.dma_start(b2t[:], wB2)

        # spin the PE while the first activations stream in, so the HAM
        # clock gate is released by the time real matmuls arrive
        warm = px.tile([126, 384], F32, tag="xq")
        for _ in range(30):
            nc.tensor.matmul(warm[:, 0:128], ident[:, 0:126], ident[:],
                             start=True, stop=True)

        for b in range(BPC):
            ps = pestage_p.tile([128, NST, T, PEK0], BF16, tag="ps")
            nc.sync.dma_start(ps[:], pe[b].rearrange("(n p) t c -> p n t c", p=128))
            hs = hstage_p.tile([128, 2, NST, T, A, 3], BF16, tag="hs")
            hsrc = [h2, h1]
            for h in range(2):
                for q in range(2):
                    nc.sync.dma_start(
                        hs[:, h, 3 * q:3 * (q + 1)],
                        hsrc[h][b, 384 * q:384 * (q + 1)].rearrange(
                            "(n p) t a c -> p n t a c", p=128))

            # ---- pe branch (per b, shared by all 16 antennas) ----
            pcp = pcp_p.tile([128, NST, PEK0, T], BF16, tag="pcp")
            nc.vector.tensor_copy(pcp[:], ps[:].rearrange("p n t c -> p n c t"))
            peT = ht_p.tile([28, SP], BF16, tag="peT")
            nc.vector.memset(peT[:], 0.0)
            for g in range(2):
                tp = ptp.tile([84, 384], BF16, tag="tp")
                for j in range(3):
                    st = g * 3 + j
                    nc.tensor.transpose(
                        tp[0:28, j * 128:(j + 1) * 128],
                        pcp[:, st], ident[:])
                nc.vector.tensor_copy(peT[0:28, 1 + g * 384:1 + (g + 1) * 384], tp[0:28, :])
            # pe_dw rows [29]: rows 0:28 = depthwise(pe), row 28 = ones
            pedw = pedw_p.tile([29, S], BF16, tag="pedw")
            nc.vector.memset(pedw[:], 1.0)
            for g, (s0, sn) in enumerate(SCH):
                dq = pdw.tile([84, 384], F32, tag="dq")
                for ds in range(3):
                    nc.tensor.matmul(dq[0:28, :sn], bpe[:, ds, :],
                                     peT[:, ds + s0: ds + s0 + sn],
                                     start=(ds == 0), stop=(ds == 2))
                nc.scalar.copy(pedw[0:28, s0:s0 + sn], dq[0:28, :sn])

            # two persistent conv1-pw rhs tiles (double-buffered by antenna
            # parity); rows 84:113 = pe branch, written once per b
            hdps = [hdp_p.tile([KPW, S], BF16, tag=f"hdp{i}", name=f"hdp{i}")
                    for i in range(2)]
            for i in range(2):
                nc.sync.dma_start(hdps[i][84:KPW, :], pedw[:])

            # persistent hT / x tiles (halos zeroed once per b; x chunk uc=3
            # has an extra all-ones row 70 feeding the conv2 bias)
            hTs_ = []
            xss_ = []
            for i in range(2):
                hT = ht_p.tile([84, SP], BF16, tag=f"hT{i}", name=f"hT{i}")
                nc.vector.memset(hT[:, 0:1], 0.0)
                nc.vector.memset(hT[:, SP - 1:SP], 0.0)
                hTs_.append(hT)
                xs = []
                for uc in range(4):
                    m = UCH[uc] * T + (1 if uc == 3 else 0)
                    xt = x_p.tile([m, SP], BF16, tag=f"x{uc}_{i}",
                                  name=f"x{uc}_{i}")
                    if uc == 3:
                        nc.vector.memset(xt[:], 1.0)
                    nc.vector.memset(xt[:, 0:1], 0.0)
                    nc.vector.memset(xt[:, SP - 1:SP], 0.0)
                    xs.append(xt)
                xss_.append(xs)

            stout = stout_p.tile([128, NST, T, A, K1], BF16, tag="so")

            # ---- per-antenna images, processed in interleaved pairs ----
            def ingest(a):
                hcp = hcp_p.tile([128, NST, 2, 3, T], BF16, tag=f"hcp{a % 2}")
                for h in range(2):
                    src = hs[:, h, :, :, a, :].rearrange("p n t c -> p n c t")
                    nc.gpsimd.tensor_copy(hcp[:, :, h], src)
                hT = hTs_[a % 2]
                for g in range(2):
                    tp = ptp.tile([84, 384], BF16, tag="tp")
                    for j in range(3):
                        st = g * 3 + j
                        nc.tensor.transpose(
                            tp[:, j * 128:(j + 1) * 128],
                            hcp[:, st], ident[:])
                    dst = hT[:, 1 + g * 384:1 + (g + 1) * 384]
                    if g == 0:
                        nc.vector.tensor_copy(dst, tp[:, :])
                    else:
                        nc.scalar.copy(dst, tp[:, :])
                return hT

            def dw1(a, hT):
                hdp = hdps[a % 2]
                for g, (s0, sn) in enumerate(SCH):
                    dq = pdw.tile([84, 384], F32, tag="dq")
                    for ds in range(3):
                        nc.tensor.matmul(dq[:, :sn], b1[:, ds, :],
                                         hT[:, ds + s0: ds + s0 + sn],
                                         start=(ds == 0), stop=(ds == 2))
                    if g == 0:
                        nc.vector.tensor_copy(hdp[0:84, s0:s0 + sn], dq[:, :sn])
                    else:
                        nc.scalar.copy(hdp[0:84, s0:s0 + sn], dq[:, :sn])
                return hdp

            def pw_relu(a, hdp):
                xs = xss_[a % 2]
                for uc in range(4):
                    m = UCH[uc] * T
                    c0 = UOF[uc] * T
                    xt = xs[uc]
                    for g, (s0, sn) in enumerate(SCH):
                        xq = px.tile([126, 384], F32, tag="xq")
                        nc.tensor.matmul(xq[0:m, :sn], w1s[:, c0:c0 + m],
                                         hdp[:, s0:s0 + sn], start=True, stop=True)
                        dst = xt[0:m, 1 + s0:1 + s0 + sn]
                        if (uc * 2 + g + a) % 2 == 0:
                            nc.scalar.activation(dst, xq[0:m, :sn], RELU)
                        else:
                            nc.vector.tensor_scalar_max(dst, xq[0:m, :sn], 0.0)
                return xs

            def conv2(a, xs):
                y2 = py.tile([128, NST, K1 * T], F32, tag="y2")
                for n in range(NST):
                    kk = 0
                    for uc in range(4):
                        m = UCH[uc] * T + (1 if uc == 3 else 0)
                        for ds in range(3):
                            c0 = (uc * 3 + ds) * (K1 * T)
                            nc.tensor.matmul(
                                y2[:, n],
                                xs[uc][0:m, n * 128 + ds: n * 128 + ds + 128],
                                b2t[0:m, c0:c0 + K1 * T],
                                start=(kk == 0), stop=(kk == 11))
                            kk += 1
                # reorder (k,t)->(t,k) into the store tile (bias already in)
                src_ = y2[:].rearrange("p n (k t) -> p n t k", k=K1)
                if a % 2 == 0:
                    nc.scalar.copy(stout[:, :, :, a, :], src_)
                else:
                    nc.vector.tensor_copy(stout[:, :, :, a, :], src_)

            for half in range(A // 2):
                pair = (2 * half, 2 * half + 1)
                hTs = {a: ingest(a) for a in pair}
                hd_ = {a: dw1(a, hTs[a]) for a in pair}
                xs_ = {a: pw_relu(a, hd_[a]) for a in pair}
                for a in pair:
                    conv2(a, xs_[a])

            yv = y[b].rearrange("(n p) t a k -> p n t a k", p=128)
            nc.sync.dma_start(yv[:, 0:2], stout[:, 0:2])
            nc.scalar.dma_start(yv[:, 2:4], stout[:, 2:4])
            nc.gpsimd.dma_start(yv[:, 4:6], stout[:, 4:6])
    nc.compile()
    return nc


_CACHED_NC = None


def get_nc():
    global _CACHED_NC
    if _CACHED_NC is None:
        _CACHED_NC = _trace_kernel(
            bacc.Bacc("TRN2", target_bir_lowering=False, debug=False))
    return _CACHED_NC


def make_in_maps(inputs):
    consts = build_consts(
        inputs["w_hh"], inputs["b_hh"], inputs["w_vh"], inputs["b_vh"],
        inputs["w_hp"], inputs["b_hp"], inputs["w_vp"], inputs["b_vp"],
        inputs["w_oh"], inputs["b_oh"], inputs["w_ov"], inputs["b_ov"])
    consts = {
        k: np.ascontiguousarray(v, NPBF16) for k, v in consts.items()
    }
    in_maps = []
    for i in range(NCORES):
        sl = slice(i * BPC, (i + 1) * BPC)
        m = {
            "h1": np.ascontiguousarray(inputs["h1"][sl], NPBF16),
            "h2": np.ascontiguousarray(inputs["h2"][sl], NPBF16),
            "pe": np.ascontiguousarray(inputs["pe"][sl], NPBF16),
        }
        m.update(consts)
        in_maps.append(m)
    return in_maps


def kernel(**inputs):
    nc = get_nc()
    in_maps = make_in_maps(inputs)
    res = run_bass_kernel_spmd(nc, in_maps, list(range(NCORES)))
    return np.concatenate([r["y"] for r in res.results], axis=0).astype(np.float32)
